# revision 1
# baseline (speedup 1.0000x reference)
"""ErrorAwareEdgeLoss Trainium2 kernel.

Math: loss = mean_b [ (sum_e w_be * P[b,i_e,:] @ D @ P[b,j_e,:]) / max(sum_e w_be, 1e-8) ]

Reformulation:
    G_b = (P_b @ D) @ P_b^T            (two 256^3 matmuls on the PE)
    sum_e w_e * P[b,i_e,:] @ D @ P[b,j_e,:] = sum_e w_e * G_b[i_e, j_e]

Per-edge access path (HW-validated primitives only):
    flat_e = 256*i_e + j_e; token t_e = flat_e >> 6; offset r_e = flat_e & 63.
    G_b spills to DRAM as a [1024, 64]-f32 token table; a single gpsimd
    dma_gather fetches all 8192 tokens (256B rows; edge e lands at partition
    e%128, slot e//128); a DVE one-hot mask over the 64 token lanes selects
    r_e, reduces, and dots with w.

Sharding: data-parallel over batch: 8 NeuronCores x 8 batches. Each core
emits a partial sum of per-sample losses; the host adds the 8 partials and
divides by B (the all-reduce of the sharding hint).
"""

from contextlib import ExitStack

import numpy as np

import concourse.bacc as bacc
import concourse.bass as bass
import concourse.mybir as mybir
import concourse.tile as tile
from concourse.bass_utils import run_bass_kernel_spmd

B, N, E = 64, 256, 8192
NCORES = 8
BPC = B // NCORES  # batches per core
Q = E // 128  # edges per partition (64)
TOK = 64  # f32 per gathered token row (256B)
NTOK = N * N // TOK  # 1024

f32 = mybir.dt.float32
bf16 = mybir.dt.bfloat16
i16 = mybir.dt.int16
i32 = mybir.dt.int32

MM_DTYPE = f32


def _build_bass():
    nc = bacc.Bacc("TRN2", target_bir_lowering=False, debug=False, num_swdge_queues=4, dynamic_dma_scratch_size=65536)

    pt_in = nc.dram_tensor("pt", [BPC, 128, 2, N], f32, kind="ExternalInput")
    d_in = nc.dram_tensor("derr", [128, 2, N], f32, kind="ExternalInput")
    ei_in = nc.dram_tensor("ei", [BPC, 128, Q], i32, kind="ExternalInput")
    ej_in = nc.dram_tensor("ej", [BPC, 128, Q], i32, kind="ExternalInput")
    ew_in = nc.dram_tensor("ew", [BPC, 128, Q], f32, kind="ExternalInput")
    ei2_in = nc.dram_tensor("ei2", [BPC, 16, E // 16], i32, kind="ExternalInput")
    ej2_in = nc.dram_tensor("ej2", [BPC, 16, E // 16], i32, kind="ExternalInput")
    out = nc.dram_tensor("out", [1, 1], f32, kind="ExternalOutput")

    with tile.TileContext(nc) as tc, ExitStack() as ctx:
        const_pool = ctx.enter_context(tc.tile_pool(name="const", bufs=1))
        pt_pool = ctx.enter_context(tc.tile_pool(name="pt", bufs=3))
        qt_pool = ctx.enter_context(tc.tile_pool(name="qt", bufs=3))
        g_pool = ctx.enter_context(tc.tile_pool(name="g", bufs=3))
        e_pool = ctx.enter_context(tc.tile_pool(name="edges", bufs=4))
        tok_pool = ctx.enter_context(tc.tile_pool(name="tok", bufs=2))
        psum_pool = ctx.enter_context(tc.tile_pool(name="ps", bufs=2, space="PSUM"))
        dram_pool = ctx.enter_context(tc.tile_pool(name="dram", bufs=4, space="DRAM"))

        # constants
        d_sb = const_pool.tile([128, 2, N], f32)
        nc.sync.dma_start(d_sb[:], d_in[:])
        ones_sb = const_pool.tile([128, 1], f32)
        nc.vector.memset(ones_sb[:], 1.0)
        # iota over the token lane: iota_bf[p, q, r] = r
        iota_bf = const_pool.tile([128, Q, TOK], bf16)
        nc.gpsimd.iota(
            iota_bf[:],
            pattern=[[0, Q], [1, TOK]],
            channel_multiplier=0,
            allow_small_or_imprecise_dtypes=True,
        )
        # replication matrix: rep16[k, m] = 1 if m % 16 == k else 0
        ia_i = const_pool.tile([16, 8, 16], i32)
        nc.gpsimd.iota(ia_i[:], pattern=[[0, 8], [1, 16]], channel_multiplier=0)
        ic_i = const_pool.tile([16, 128], i32)
        nc.gpsimd.iota(ic_i[:], pattern=[[0, 128]], channel_multiplier=1)
        ia_f = const_pool.tile([16, 128], f32)
        nc.vector.tensor_copy(ia_f[:], ia_i[:].rearrange("k a b -> k (a b)"))
        ic_f = const_pool.tile([16, 128], f32)
        nc.vector.tensor_copy(ic_f[:], ic_i[:])
        rep16 = const_pool.tile([16, 128], f32)
        nc.vector.tensor_tensor(
            out=rep16[:], in0=ia_f[:], in1=ic_f[:], op=mybir.AluOpType.is_equal
        )
        # per-batch partials: cols [0,BPC) = sum(w*g), cols [BPC,2*BPC) = sum(w)
        red_sb = const_pool.tile([128, 2 * BPC], f32)

        d_mm = d_sb[:].bitcast(MM_DTYPE)

        for b in range(BPC):
            # ---- load P^T: pt_sb[p, c, i] = P^T[c*128+p, i]
            pt_sb = pt_pool.tile([128, 2, N], f32)
            nc.sync.dma_start(pt_sb[:], pt_in[b])
            pt_mm = pt_sb[:].bitcast(MM_DTYPE)

            # ---- QT = (P @ D)^T : QT[n, i] = sum_k D[k, n] * PT[k, i]
            qt_sb = qt_pool.tile([128, 2, N], f32)
            for ncx in range(2):
                qt_ps = psum_pool.tile([128, N], f32, tag="qtps")
                for kc in range(2):
                    nc.tensor.matmul(
                        qt_ps[:],
                        lhsT=d_mm[:, kc, ncx * 128 : (ncx + 1) * 128],
                        rhs=pt_mm[:, kc, :],
                        start=(kc == 0),
                        stop=(kc == 1),
                    )
                nc.scalar.copy(qt_sb[:, ncx, :], qt_ps[:])
            qt_mm = qt_sb[:].bitcast(MM_DTYPE)

            # ---- G = Q @ P^T : G[i, j] = sum_n QT[n, i] * PT[n, j]
            g_sb = g_pool.tile([128, 2, N], f32)
            for ic in range(2):
                g_ps = psum_pool.tile([128, N], f32, tag="gps")
                for ncx in range(2):
                    nc.tensor.matmul(
                        g_ps[:],
                        lhsT=qt_mm[:, ncx, ic * 128 : (ic + 1) * 128],
                        rhs=pt_mm[:, ncx, :],
                        start=(ncx == 0),
                        stop=(ncx == 1),
                    )
                nc.scalar.copy(g_sb[:, ic, :], g_ps[:])

            # ---- spill G to DRAM; g_d natural (c,p,j) order == G_flat order
            g_d = dram_pool.tile([2, 128, N], f32, tag="gd")
            nc.sync.dma_start(g_d.rearrange("c p j -> p c j"), g_sb[:])

            # ---- edges (host lays edge e=q*128+p at [p, q])
            ei_sb = e_pool.tile([128, Q], i32, tag="ei")
            ej_sb = e_pool.tile([128, Q], i32, tag="ej")
            ew_sb = e_pool.tile([128, Q], f32, tag="ew")
            nc.sync.dma_start(ei_sb[:], ei_in[b])
            nc.sync.dma_start(ej_sb[:], ej_in[b])
            nc.sync.dma_start(ew_sb[:], ew_in[b])

            # r = ej mod 64 in the [p, q] layout (flat = 256*ei + ej)
            ejf = e_pool.tile([128, Q], f32, tag="ejf")
            nc.vector.tensor_copy(ejf[:], ej_sb[:])
            # h = floor(ej/64) = (ej>=64)+(ej>=128)+(ej>=192); r = ej - 64*h
            s1 = e_pool.tile([128, Q], f32, tag="s1")
            nc.vector.tensor_scalar(
                out=s1[:], in0=ejf[:], scalar1=64.0, scalar2=None,
                op0=mybir.AluOpType.is_ge,
            )
            s2 = e_pool.tile([128, Q], f32, tag="s2")
            nc.vector.scalar_tensor_tensor(
                out=s2[:], in0=ejf[:], scalar=128.0, in1=s1[:],
                op0=mybir.AluOpType.is_ge, op1=mybir.AluOpType.add,
            )
            s3 = e_pool.tile([128, Q], f32, tag="s3")
            nc.vector.scalar_tensor_tensor(
                out=s3[:], in0=ejf[:], scalar=192.0, in1=s2[:],
                op0=mybir.AluOpType.is_ge, op1=mybir.AluOpType.add,
            )
            rf = e_pool.tile([128, Q], f32, tag="rf")
            nc.vector.scalar_tensor_tensor(
                out=rf[:], in0=s3[:], scalar=-64.0, in1=ejf[:],
                op0=mybir.AluOpType.mult, op1=mybir.AluOpType.add,
            )
            rb = e_pool.tile([128, Q], bf16, tag="rb")
            nc.vector.tensor_copy(rb[:], rf[:])

            # token index t = 4*ei + (ej - ej mod 64)/64, computed directly in
            # the dma_gather wrapped layout [16, E/16] (k = s*16+pp at [pp,s])
            ei2_sb = e_pool.tile([16, E // 16], i32, tag="ei2")
            ej2_sb = e_pool.tile([16, E // 16], i32, tag="ej2")
            nc.sync.dma_start(ei2_sb[:], ei2_in[b])
            nc.sync.dma_start(ej2_sb[:], ej2_in[b])
            ei2f = e_pool.tile([16, E // 16], f32, tag="ei2f")
            ej2f = e_pool.tile([16, E // 16], f32, tag="ej2f")
            nc.vector.tensor_copy(ei2f[:], ei2_sb[:])
            nc.vector.tensor_copy(ej2f[:], ej2_sb[:])
            u1 = e_pool.tile([16, E // 16], f32, tag="u1")
            nc.vector.tensor_scalar(
                out=u1[:], in0=ej2f[:], scalar1=64.0, scalar2=None,
                op0=mybir.AluOpType.is_ge,
            )
            u2 = e_pool.tile([16, E // 16], f32, tag="u2")
            nc.vector.scalar_tensor_tensor(
                out=u2[:], in0=ej2f[:], scalar=128.0, in1=u1[:],
                op0=mybir.AluOpType.is_ge, op1=mybir.AluOpType.add,
            )
            u3 = e_pool.tile([16, E // 16], f32, tag="u3")
            nc.vector.scalar_tensor_tensor(
                out=u3[:], in0=ej2f[:], scalar=192.0, in1=u2[:],
                op0=mybir.AluOpType.is_ge, op1=mybir.AluOpType.add,
            )
            t3 = e_pool.tile([16, E // 16], f32, tag="t3")
            nc.vector.scalar_tensor_tensor(
                out=t3[:], in0=ei2f[:], scalar=4.0, in1=u3[:],
                op0=mybir.AluOpType.mult, op1=mybir.AluOpType.add,
            )

            # replicate [16, E/16] -> [128, E/16] via PE, cast to i16
            rep_ps = psum_pool.tile([128, E // 16], f32, tag="repps")
            nc.tensor.matmul(
                rep_ps[:], lhsT=rep16[:], rhs=t3[:], start=True, stop=True
            )
            ti = e_pool.tile([128, E // 16], i16, tag="ti")
            nc.vector.tensor_copy(ti[:], rep_ps[:])

            # ---- gather all 8192 tokens: tok[p, q, :] = table[t_{q*128+p}]
            # (two halves: 8192 descriptors exceed the SWDGE ring carveout)
            tok = tok_pool.tile([128, Q, TOK], f32, tag="tok")
            tab_ap = g_d.rearrange("c p (t u) -> (c p t) u", u=TOK)
            CH = 1024  # SWDGE ring holds ~1024 descriptors per instruction
            for h in range(E // CH):
                nc.gpsimd.dma_gather(
                    out_ap=tok[:, (CH // 128) * h : (CH // 128) * (h + 1), :],
                    in_ap=tab_ap,
                    idxs_ap=ti[:, (CH // 16) * h : (CH // 16) * (h + 1)],
                    num_idxs=CH,
                    num_idxs_reg=CH,
                    elem_size=TOK,
                    single_packet=False,
                    queue_num=h % 4,
                )

            # ---- select lane r: mask = (iota == r); g_sel = sum_r mask*tok
            mask = tok_pool.tile([128, Q, TOK], bf16, tag="mask")
            nc.vector.tensor_tensor(
                out=mask[:],
                in0=iota_bf[:],
                in1=rb[:].unsqueeze(-1).broadcast_to([128, Q, TOK]),
                op=mybir.AluOpType.is_equal,
            )
            nc.vector.tensor_tensor(
                out=tok[:], in0=tok[:], in1=mask[:], op=mybir.AluOpType.mult
            )
            gsel = e_pool.tile([128, Q], f32, tag="gsel")
            nc.vector.tensor_reduce(
                out=gsel[:],
                in_=tok[:],
                axis=mybir.AxisListType.X,
                op=mybir.AluOpType.add,
            )

            # ---- per-batch partial sums
            prod = e_pool.tile([128, Q], f32, tag="prod")
            nc.vector.tensor_tensor(
                out=prod[:], in0=gsel[:], in1=ew_sb[:], op=mybir.AluOpType.mult
            )
            nc.vector.tensor_reduce(
                out=red_sb[:, b : b + 1],
                in_=prod[:],
                axis=mybir.AxisListType.X,
                op=mybir.AluOpType.add,
            )
            nc.vector.tensor_reduce(
                out=red_sb[:, BPC + b : BPC + b + 1],
                in_=ew_sb[:],
                axis=mybir.AxisListType.X,
                op=mybir.AluOpType.add,
            )

        # ---- cross-partition reduce of all partials in one matmul
        red_ps = psum_pool.tile([1, 2 * BPC], f32, tag="redps")
        nc.tensor.matmul(
            red_ps[:], lhsT=ones_sb[:], rhs=red_sb[:], start=True, stop=True
        )
        fin = const_pool.tile([1, 2 * BPC], f32)
        nc.vector.tensor_copy(fin[:], red_ps[:])

        # loss_b = sl_b / max(sw_b, 1e-8); out = sum_b loss_b
        sw_cl = const_pool.tile([1, BPC], f32)
        nc.vector.tensor_scalar_max(sw_cl[:], fin[:, BPC:], 1e-8)
        rsw = const_pool.tile([1, BPC], f32)
        nc.vector.reciprocal(rsw[:], sw_cl[:])
        lb = const_pool.tile([1, BPC], f32)
        nc.vector.tensor_tensor(
            out=lb[:], in0=fin[:, :BPC], in1=rsw[:], op=mybir.AluOpType.mult
        )
        tot = const_pool.tile([1, 1], f32)
        nc.vector.tensor_reduce(
            out=tot[:], in_=lb[:], axis=mybir.AxisListType.X, op=mybir.AluOpType.add
        )
        nc.sync.dma_start(out[:], tot[:])

    if not nc.is_finalized():
        nc.finalize()
    return nc


_NC_CACHE = {}


def _get_nc():
    if "nc" not in _NC_CACHE:
        _NC_CACHE["nc"] = _build_bass()
    return _NC_CACHE["nc"]


def _prep_in_maps(P, d_error, edge_i, edge_j, edge_w):
    P = np.asarray(P, dtype=np.float32)
    d_error = np.asarray(d_error, dtype=np.float32)
    edge_i = np.asarray(edge_i, dtype=np.int32)
    edge_j = np.asarray(edge_j, dtype=np.int32)
    edge_w = np.asarray(edge_w, dtype=np.float32)

    # P^T per batch, laid out [128, 2, N]: pt[b, p, c, :] = P[b, :, c*128+p]
    PT = np.ascontiguousarray(np.transpose(P, (0, 2, 1)))  # [B, N(k), N(i)]
    PT = np.ascontiguousarray(PT.reshape(B, 2, 128, N).transpose(0, 2, 1, 3))
    D = np.ascontiguousarray(d_error.reshape(2, 128, N).transpose(1, 0, 2))

    # edge order: edge e = q*128 + p lives at [p, q]
    def lay(a):
        return np.ascontiguousarray(a.reshape(B, Q, 128).transpose(0, 2, 1))

    ei_l, ej_l, ew_l = lay(edge_i), lay(edge_j), lay(edge_w)

    # wrapped layout for the gather ucode: index k = s*16+pp at [pp, s]
    def lay2(a):
        return np.ascontiguousarray(a.reshape(B, E // 16, 16).transpose(0, 2, 1))

    ei2_l, ej2_l = lay2(edge_i), lay2(edge_j)

    in_maps = []
    for c in range(NCORES):
        sl = slice(c * BPC, (c + 1) * BPC)
        in_maps.append(
            {
                "pt": np.ascontiguousarray(PT[sl]),
                "derr": D,
                "ei": np.ascontiguousarray(ei_l[sl]),
                "ej": np.ascontiguousarray(ej_l[sl]),
                "ew": np.ascontiguousarray(ew_l[sl]),
                "ei2": np.ascontiguousarray(ei2_l[sl]),
                "ej2": np.ascontiguousarray(ej2_l[sl]),
            }
        )
    return in_maps


def run(P, d_error, edge_i, edge_j, edge_w, trace=False):
    """Run on 8 cores; returns (loss_scalar, BassKernelResults)."""
    nc = _get_nc()
    in_maps = _prep_in_maps(P, d_error, edge_i, edge_j, edge_w)
    res = run_bass_kernel_spmd(
        nc, in_maps, core_ids=list(range(NCORES)), trace=trace
    )
    partials = [r["out"].reshape(()) for r in res.results]
    loss = np.float32(np.sum(np.stack(partials), dtype=np.float64) / B)
    return loss, res


def kernel(P, d_error, edge_i, edge_j, edge_w):
    loss, _ = run(P, d_error, edge_i, edge_j, edge_w, trace=False)
    return np.asarray(loss, dtype=np.float32)



# revision 2
# speedup vs baseline: 6.8024x; 6.8024x over previous
"""ErrorAwareEdgeLoss Trainium2 kernel.

Math: loss = mean_b [ (sum_e w_be * P[b,i_e,:] @ D @ P[b,j_e,:]) / max(sum_e w_be, 1e-8) ]

Reformulation:
    G_b = (P_b @ D) @ P_b^T            (two 256^3 matmuls on the PE, bf16)
    sum_e w_e * P[b,i_e,:] @ D @ P[b,j_e,:] = sum_e w_e * G_b[i_e, j_e]

Per-edge access path:
    G_b spills to DRAM as a flat [65536]-f32 table; one indirect DMA
    (hardware dynamic DGE, per-element offsets f_e = 256*i_e + j_e read
    from SBUF) fetches all 8192 per-edge values directly — no token
    expansion, no mask/select.

Sharding: data-parallel over batch: 8 NeuronCores x 8 batches. Each core
emits a partial sum of per-sample losses; the host adds the 8 partials and
divides by B (the all-reduce of the sharding hint).
"""

from contextlib import ExitStack

import ml_dtypes
import numpy as np

import concourse.bacc as bacc
import concourse.bass as bass
import concourse.mybir as mybir
import concourse.tile as tile
from concourse.bass_utils import run_bass_kernel_spmd

B, N, E = 64, 256, 8192
NCORES = 8
BPC = B // NCORES  # batches per core
Q = E // 128  # edges per partition (64)

f32 = mybir.dt.float32
bf16 = mybir.dt.bfloat16
i32 = mybir.dt.int32


def _build_bass():
    nc = bacc.Bacc("TRN2", target_bir_lowering=False, debug=False)

    pt_in = nc.dram_tensor("pt", [BPC, 128, 2, N], bf16, kind="ExternalInput")
    d_in = nc.dram_tensor("derr", [128, 2, N], bf16, kind="ExternalInput")
    fi_in = nc.dram_tensor("fidx", [BPC, 128, Q], i32, kind="ExternalInput")
    ew_in = nc.dram_tensor("ew", [BPC, 128, Q], f32, kind="ExternalInput")
    out = nc.dram_tensor("out", [1, 1], f32, kind="ExternalOutput")

    with tile.TileContext(nc) as tc, ExitStack() as ctx:
        const_pool = ctx.enter_context(tc.tile_pool(name="const", bufs=1))
        pt_pool = ctx.enter_context(tc.tile_pool(name="pt", bufs=3))
        qt_pool = ctx.enter_context(tc.tile_pool(name="qt", bufs=3))
        g_pool = ctx.enter_context(tc.tile_pool(name="g", bufs=3))
        e_pool = ctx.enter_context(tc.tile_pool(name="edges", bufs=4))
        psum_pool = ctx.enter_context(tc.tile_pool(name="ps", bufs=2, space="PSUM"))
        dram_pool = ctx.enter_context(tc.tile_pool(name="dram", bufs=3, space="DRAM"))

        # constants
        d_sb = const_pool.tile([128, 2, N], bf16)
        nc.sync.dma_start(d_sb[:], d_in[:])
        ones_sb = const_pool.tile([128, 1], f32)
        nc.vector.memset(ones_sb[:], 1.0)
        # per-batch partials: cols [0,BPC) = sum(w*g), cols [BPC,2*BPC) = sum(w)
        red_sb = const_pool.tile([128, 2 * BPC], f32)

        for b in range(BPC):
            # ---- load P^T: pt_sb[p, c, i] = P^T[c*128+p, i]  (bf16)
            pt_sb = pt_pool.tile([128, 2, N], bf16)
            nc.sync.dma_start(pt_sb[:], pt_in[b])

            # ---- QT = (P @ D)^T : QT[n, i] = sum_k D[k, n] * PT[k, i]
            qt_sb = qt_pool.tile([128, 2, N], bf16)
            for ncx in range(2):
                qt_ps = psum_pool.tile([128, N], f32, tag="qtps")
                for kc in range(2):
                    nc.tensor.matmul(
                        qt_ps[:],
                        lhsT=d_sb[:, kc, ncx * 128 : (ncx + 1) * 128],
                        rhs=pt_sb[:, kc, :],
                        start=(kc == 0),
                        stop=(kc == 1),
                    )
                nc.scalar.copy(qt_sb[:, ncx, :], qt_ps[:])

            # ---- G = Q @ P^T : G[i, j] = sum_n QT[n, i] * PT[n, j]  (f32 out)
            g_sb = g_pool.tile([128, 2, N], f32)
            for ic in range(2):
                g_ps = psum_pool.tile([128, N], f32, tag="gps")
                for ncx in range(2):
                    nc.tensor.matmul(
                        g_ps[:],
                        lhsT=qt_sb[:, ncx, ic * 128 : (ic + 1) * 128],
                        rhs=pt_sb[:, ncx, :],
                        start=(ncx == 0),
                        stop=(ncx == 1),
                    )
                nc.scalar.copy(g_sb[:, ic, :], g_ps[:])

            # ---- spill G to DRAM; (c, p, j) order == flat f = 256*i + j order
            g_d = dram_pool.tile([2, 128, N], f32, tag="gd")
            nc.sync.dma_start(g_d.rearrange("c p j -> p c j"), g_sb[:])

            # ---- edges (host lays edge e=q*128+p at [p, q]; f = 256*i + j)
            f_sb = e_pool.tile([128, Q], i32, tag="fi")
            ew_sb = e_pool.tile([128, Q], f32, tag="ew")
            nc.sync.dma_start(f_sb[:], fi_in[b])
            nc.sync.dma_start(ew_sb[:], ew_in[b])

            # ---- gather all 8192 per-edge values in one indirect DMA
            gsel = e_pool.tile([128, Q], f32, tag="gsel")
            nc.gpsimd.indirect_dma_start(
                out=gsel[:],
                out_offset=None,
                in_=g_d.rearrange("c p (j u) -> (c p j) u", u=1),
                in_offset=bass.IndirectOffsetOnAxis(ap=f_sb[:], axis=0),
            )

            # ---- per-batch partial sums
            prod = e_pool.tile([128, Q], f32, tag="prod")
            nc.vector.tensor_tensor(
                out=prod[:], in0=gsel[:], in1=ew_sb[:], op=mybir.AluOpType.mult
            )
            nc.vector.tensor_reduce(
                out=red_sb[:, b : b + 1],
                in_=prod[:],
                axis=mybir.AxisListType.X,
                op=mybir.AluOpType.add,
            )
            nc.vector.tensor_reduce(
                out=red_sb[:, BPC + b : BPC + b + 1],
                in_=ew_sb[:],
                axis=mybir.AxisListType.X,
                op=mybir.AluOpType.add,
            )

        # ---- cross-partition reduce of all partials in one matmul
        red_ps = psum_pool.tile([1, 2 * BPC], f32, tag="redps")
        nc.tensor.matmul(
            red_ps[:], lhsT=ones_sb[:], rhs=red_sb[:], start=True, stop=True
        )
        fin = const_pool.tile([1, 2 * BPC], f32)
        nc.vector.tensor_copy(fin[:], red_ps[:])

        # loss_b = sl_b / max(sw_b, 1e-8); out = sum_b loss_b
        sw_cl = const_pool.tile([1, BPC], f32)
        nc.vector.tensor_scalar_max(sw_cl[:], fin[:, BPC:], 1e-8)
        rsw = const_pool.tile([1, BPC], f32)
        nc.vector.reciprocal(rsw[:], sw_cl[:])
        lb = const_pool.tile([1, BPC], f32)
        nc.vector.tensor_tensor(
            out=lb[:], in0=fin[:, :BPC], in1=rsw[:], op=mybir.AluOpType.mult
        )
        tot = const_pool.tile([1, 1], f32)
        nc.vector.tensor_reduce(
            out=tot[:], in_=lb[:], axis=mybir.AxisListType.X, op=mybir.AluOpType.add
        )
        nc.sync.dma_start(out[:], tot[:])

    if not nc.is_finalized():
        nc.finalize()
    return nc


_NC_CACHE = {}


def _get_nc():
    if "nc" not in _NC_CACHE:
        _NC_CACHE["nc"] = _build_bass()
    return _NC_CACHE["nc"]


def _prep_in_maps(P, d_error, edge_i, edge_j, edge_w):
    P = np.asarray(P, dtype=np.float32)
    d_error = np.asarray(d_error, dtype=np.float32)
    edge_i = np.asarray(edge_i, dtype=np.int32)
    edge_j = np.asarray(edge_j, dtype=np.int32)
    edge_w = np.asarray(edge_w, dtype=np.float32)

    # P^T per batch, laid out [128, 2, N]: pt[b, p, c, :] = P[b, :, c*128+p]
    PT = np.ascontiguousarray(np.transpose(P, (0, 2, 1)))  # [B, N(k), N(i)]
    PT = PT.reshape(B, 2, 128, N).transpose(0, 2, 1, 3)
    PT = np.ascontiguousarray(PT).astype(ml_dtypes.bfloat16)
    D = np.ascontiguousarray(d_error.reshape(2, 128, N).transpose(1, 0, 2))
    D = D.astype(ml_dtypes.bfloat16)

    # flat gather index f = 256*i + j; edge order: edge e = q*128 + p at [p, q]
    fidx = (edge_i << 8) | edge_j  # [B, E] int32

    def lay(a):
        return np.ascontiguousarray(a.reshape(B, Q, 128).transpose(0, 2, 1))

    fi_l, ew_l = lay(fidx), lay(edge_w)

    in_maps = []
    for c in range(NCORES):
        sl = slice(c * BPC, (c + 1) * BPC)
        in_maps.append(
            {
                "pt": np.ascontiguousarray(PT[sl]),
                "derr": D,
                "fidx": np.ascontiguousarray(fi_l[sl]),
                "ew": np.ascontiguousarray(ew_l[sl]),
            }
        )
    return in_maps


def run(P, d_error, edge_i, edge_j, edge_w, trace=False):
    """Run on 8 cores; returns (loss_scalar, BassKernelResults)."""
    nc = _get_nc()
    in_maps = _prep_in_maps(P, d_error, edge_i, edge_j, edge_w)
    res = run_bass_kernel_spmd(
        nc, in_maps, core_ids=list(range(NCORES)), trace=trace
    )
    partials = [r["out"].reshape(()) for r in res.results]
    loss = np.float32(np.sum(np.stack(partials), dtype=np.float64) / B)
    return loss, res


def kernel(P, d_error, edge_i, edge_j, edge_w):
    loss, _ = run(P, d_error, edge_i, edge_j, edge_w, trace=False)
    return np.asarray(loss, dtype=np.float32)


# revision 5
# speedup vs baseline: 7.1128x; 1.0456x over previous
"""ErrorAwareEdgeLoss Trainium2 kernel.

Math: loss = mean_b [ (sum_e w_be * P[b,i_e,:] @ D @ P[b,j_e,:]) / max(sum_e w_be, 1e-8) ]

Reformulation:
    G_b = (P_b @ D) @ P_b^T            (two 256^3 matmuls on the PE, bf16)
    sum_e w_e * P[b,i_e,:] @ D @ P[b,j_e,:] = sum_e w_e * G_b[i_e, j_e]

Per-edge access path:
    G_b spills to DRAM as a flat [65536]-f32 table; one indirect DMA
    (hardware dynamic DGE, per-element offsets f_e = 256*i_e + j_e read
    from SBUF) fetches all 8192 per-edge values directly.

Batch pairing: QT = (P @ D)^T is computed for two batches per matmul
(rhs 512 wide) to halve PE instruction count; all edge indices/weights
load in one DMA up front; G spills per batch feed per-batch gathers.

Sharding: data-parallel over batch: 8 NeuronCores x 8 batches. Each core
emits per-sample partial sums (sum w*g and sum w per batch); the host
performs the final divide + mean (the all-reduce of the sharding hint).
"""

from contextlib import ExitStack

import ml_dtypes
import numpy as np

import concourse.bacc as bacc
import concourse.bass as bass
import concourse.mybir as mybir
import concourse.tile as tile
from concourse.bass_utils import run_bass_kernel_spmd

B, N, E = 64, 256, 8192
NCORES = 8
BPC = B // NCORES  # batches per core
NPAIR = BPC // 2
Q = E // 128  # edges per partition (64)

f32 = mybir.dt.float32
bf16 = mybir.dt.bfloat16
i32 = mybir.dt.int32


def _build_bass():
    nc = bacc.Bacc("TRN2", target_bir_lowering=False, debug=False)

    # pt[t, p, kc, b2, i] = P[2t+b2, i, kc*128+p]
    pt_in = nc.dram_tensor("pt", [NPAIR, 128, 2, 2, N], bf16, kind="ExternalInput")
    d_in = nc.dram_tensor("derr", [128, 2, N], bf16, kind="ExternalInput")
    # edges[p, b, 0, q] = 256*i + j; edges[p, b, 1, q] = bits(w); edge e=q*128+p
    e_in = nc.dram_tensor("edges", [128, BPC, 2, Q], i32, kind="ExternalInput")
    out = nc.dram_tensor("out", [1, 2 * BPC], f32, kind="ExternalOutput")

    with tile.TileContext(nc) as tc, ExitStack() as ctx:
        const_pool = ctx.enter_context(tc.tile_pool(name="const", bufs=1))
        pt_pool = ctx.enter_context(tc.tile_pool(name="pt", bufs=2))
        qt_pool = ctx.enter_context(tc.tile_pool(name="qt", bufs=2))
        g_pool = ctx.enter_context(tc.tile_pool(name="g", bufs=3))
        e_pool = ctx.enter_context(tc.tile_pool(name="edges", bufs=4))
        psum_pool = ctx.enter_context(tc.tile_pool(name="ps", bufs=3, space="PSUM"))
        psum1_pool = ctx.enter_context(tc.tile_pool(name="ps1", bufs=1, space="PSUM"))
        dram_pool = ctx.enter_context(tc.tile_pool(name="dram", bufs=3, space="DRAM"))

        # constants
        d_sb = const_pool.tile([128, 2, N], bf16)
        nc.sync.dma_start(d_sb[:], d_in[:])
        edges_sb = const_pool.tile([128, BPC, 2, Q], i32)
        nc.sync.dma_start(edges_sb[:], e_in[:])
        ones_sb = const_pool.tile([128, 1], f32)
        nc.vector.memset(ones_sb[:], 1.0)
        # per-batch partials: cols [0,BPC) = sum(w*g), cols [BPC,2*BPC) = sum(w)
        red_sb = const_pool.tile([128, 2 * BPC], f32)

        for t in range(NPAIR):
            # ---- load P^T for a batch pair
            pt2 = pt_pool.tile([128, 2, 2, N], bf16)
            nc.sync.dma_start(pt2[:], pt_in[t])

            # ---- QT[n, (b2, i)] = sum_k D[k, n] * PT[k, (b2, i)]
            qt_sb = qt_pool.tile([128, 2, 2, N], bf16)  # (ncx, b2, i)
            for ncx in range(2):
                qt_ps = psum_pool.tile([128, 2, N], f32, tag="qtps")
                for kc in range(2):
                    nc.tensor.matmul(
                        qt_ps[:].rearrange("p a b -> p (a b)"),
                        lhsT=d_sb[:, kc, ncx * 128 : (ncx + 1) * 128],
                        rhs=pt2[:, kc, :, :].rearrange("p a b -> p (a b)"),
                        start=(kc == 0),
                        stop=(kc == 1),
                    )
                nc.scalar.copy(qt_sb[:, ncx], qt_ps[:])

            for b2 in range(2):
                b = 2 * t + b2
                # ---- G[(ic), j] = sum_n QT[n, i] * PT[n, j]
                g_ps = psum_pool.tile([128, 2, N], f32, tag="gps")  # (ic, j)
                for ic in range(2):
                    for ncx in range(2):
                        nc.tensor.matmul(
                            g_ps[:, ic, :],
                            lhsT=qt_sb[:, ncx, b2, ic * 128 : (ic + 1) * 128],
                            rhs=pt2[:, ncx, b2, :],
                            start=(ncx == 0),
                            stop=(ncx == 1),
                        )
                g_sb = g_pool.tile([128, 2, N], f32)
                if b2 == 0:
                    nc.vector.tensor_copy(g_sb[:], g_ps[:])
                else:
                    nc.scalar.copy(g_sb[:], g_ps[:])

                # ---- spill G; (c, p, j) order == flat f = 256*i + j order
                g_d = dram_pool.tile([2, 128, N], f32, tag="gd")
                nc.sync.dma_start(g_d.rearrange("c p j -> p c j"), g_sb[:])

                # ---- gather all 8192 per-edge values in one indirect DMA
                gsel = e_pool.tile([128, Q], f32, tag="gsel")
                nc.gpsimd.indirect_dma_start(
                    out=gsel[:],
                    out_offset=None,
                    in_=g_d.rearrange("c p (j u) -> (c p j) u", u=1),
                    in_offset=bass.IndirectOffsetOnAxis(
                        ap=edges_sb[:, b, 0, :], axis=0
                    ),
                )

                # ---- per-batch partial sums
                ew_ap = edges_sb[:, b, 1, :].bitcast(f32)
                prod = e_pool.tile([128, Q], f32, tag="prod")
                nc.vector.tensor_tensor(
                    out=prod[:], in0=gsel[:], in1=ew_ap, op=mybir.AluOpType.mult
                )
                nc.vector.tensor_reduce(
                    out=red_sb[:, b : b + 1],
                    in_=prod[:],
                    axis=mybir.AxisListType.X,
                    op=mybir.AluOpType.add,
                )
                nc.vector.tensor_reduce(
                    out=red_sb[:, BPC + b : BPC + b + 1],
                    in_=ew_ap,
                    axis=mybir.AxisListType.X,
                    op=mybir.AluOpType.add,
                )

        # ---- cross-partition reduce of all partials in one matmul
        red_ps = psum1_pool.tile([1, 2 * BPC], f32, tag="redps")
        nc.tensor.matmul(
            red_ps[:], lhsT=ones_sb[:], rhs=red_sb[:], start=True, stop=True
        )
        fin = const_pool.tile([1, 2 * BPC], f32)
        nc.vector.tensor_copy(fin[:], red_ps[:])
        nc.sync.dma_start(out[:], fin[:])

    if not nc.is_finalized():
        nc.finalize()
    return nc


_NC_CACHE = {}


def _get_nc():
    if "nc" not in _NC_CACHE:
        _NC_CACHE["nc"] = _build_bass()
    return _NC_CACHE["nc"]


def _prep_in_maps(P, d_error, edge_i, edge_j, edge_w):
    P = np.asarray(P, dtype=np.float32)
    d_error = np.asarray(d_error, dtype=np.float32)
    edge_i = np.asarray(edge_i, dtype=np.int32)
    edge_j = np.asarray(edge_j, dtype=np.int32)
    edge_w = np.asarray(edge_w, dtype=np.float32)

    # P^T pairs: pt[t, p, kc, b2, i] = P[2t+b2, i, kc*128+p]
    PT = np.ascontiguousarray(np.transpose(P, (0, 2, 1)))  # [B, N(k), N(i)]
    PT = PT.reshape(B // 2, 2, 2, 128, N).transpose(0, 3, 2, 1, 4)
    PT = np.ascontiguousarray(PT).astype(ml_dtypes.bfloat16)
    D = np.ascontiguousarray(d_error.reshape(2, 128, N).transpose(1, 0, 2))
    D = D.astype(ml_dtypes.bfloat16)

    # packed edges: [p, b, {fidx, w-bits}, q]; edge e = q*128 + p at [p, q]
    fidx = (edge_i << 8) | edge_j  # [B, E] int32
    wbits = edge_w.view(np.int32)
    packed = np.stack([fidx, wbits], axis=1)  # [B, 2, E]
    packed = packed.reshape(B, 2, Q, 128).transpose(3, 0, 1, 2)  # [128, B, 2, Q]
    packed = np.ascontiguousarray(packed)

    in_maps = []
    for c in range(NCORES):
        in_maps.append(
            {
                "pt": np.ascontiguousarray(PT[c * NPAIR : (c + 1) * NPAIR]),
                "derr": D,
                "edges": np.ascontiguousarray(
                    packed[:, c * BPC : (c + 1) * BPC]
                ),
            }
        )
    return in_maps


def run(P, d_error, edge_i, edge_j, edge_w, trace=False):
    """Run on 8 cores; returns (loss_scalar, BassKernelResults)."""
    nc = _get_nc()
    in_maps = _prep_in_maps(P, d_error, edge_i, edge_j, edge_w)
    res = run_bass_kernel_spmd(
        nc, in_maps, core_ids=list(range(NCORES)), trace=trace
    )
    # host-side all-reduce: loss = mean_b( sl_b / max(sw_b, 1e-8) )
    acc = 0.0
    for r in res.results:
        part = np.asarray(r["out"], dtype=np.float64).reshape(2 * BPC)
        sl, sw = part[:BPC], part[BPC:]
        acc += float(np.sum(sl / np.maximum(sw, 1e-8)))
    loss = np.float32(acc / B)
    return loss, res


def kernel(P, d_error, edge_i, edge_j, edge_w):
    loss, _ = run(P, d_error, edge_i, edge_j, edge_w, trace=False)
    return np.asarray(loss, dtype=np.float32)


# revision 12
# speedup vs baseline: 7.4242x; 1.0438x over previous
"""ErrorAwareEdgeLoss Trainium2 kernel.

Math: loss = mean_b [ (sum_e w_be * P[b,i_e,:] @ D @ P[b,j_e,:]) / max(sum_e w_be, 1e-8) ]

Reformulation:
    G_b = (P_b @ D) @ P_b^T            (two 256^3 matmuls on the PE, bf16)
    sum_e w_e * P[b,i_e,:] @ D @ P[b,j_e,:] = sum_e w_e * G_b[i_e, j_e]

Per-edge access path:
    G spills to DRAM as a flat f32 table (two batches per table); one
    indirect DMA per batch pair (hardware dynamic DGE, per-element offsets
    f = b2*65536 + 256*i + j read from SBUF) fetches all 16384 per-edge
    values directly.

Batch pairing: QT = (P @ D)^T is computed for two batches per matmul
(rhs 512 wide) to halve PE instruction count; all edge indices/weights
load in one DMA up front; one spill + one gather per pair.

Sharding: data-parallel over batch: 8 NeuronCores x 8 batches. Each core
emits per-sample partial sums (sum w*g and sum w per batch); the host
performs the final divide + mean (the all-reduce of the sharding hint).
"""

from contextlib import ExitStack

import ml_dtypes
import numpy as np

import concourse.bacc as bacc
import concourse.bass as bass
import concourse.mybir as mybir
import concourse.tile as tile
from concourse.bass_utils import run_bass_kernel_spmd

B, N, E = 64, 256, 8192
NCORES = 8
BPC = B // NCORES  # batches per core
NPAIR = BPC // 2
Q = E // 128  # edges per partition (64)

f32 = mybir.dt.float32
bf16 = mybir.dt.bfloat16
i32 = mybir.dt.int32


def _build_bass():
    nc = bacc.Bacc("TRN2", target_bir_lowering=False, debug=False)

    # pt[t, p, kc, b2, i] = P[2t+b2, i, kc*128+p]
    pt_in = nc.dram_tensor("pt", [NPAIR, 128, 2, 2, N], bf16, kind="ExternalInput")
    d_in = nc.dram_tensor("derr", [128, 2, N], bf16, kind="ExternalInput")
    # edges[p, t, 0, b2, q] = (b%2)*65536 + 256*i + j; edges[p, t, 1, b2, q] = bits(w)
    e_in = nc.dram_tensor("edges", [128, NPAIR, 2, 2, Q], i32, kind="ExternalInput")
    out = nc.dram_tensor("out", [1, 2 * BPC], f32, kind="ExternalOutput")

    with tile.TileContext(nc) as tc, ExitStack() as ctx:
        const_pool = ctx.enter_context(tc.tile_pool(name="const", bufs=1))
        pt_pool = ctx.enter_context(tc.tile_pool(name="pt", bufs=3))
        qt_pool = ctx.enter_context(tc.tile_pool(name="qt", bufs=2))
        g_pool = ctx.enter_context(tc.tile_pool(name="g", bufs=2))
        e_pool = ctx.enter_context(tc.tile_pool(name="edges", bufs=3))
        psum_pool = ctx.enter_context(tc.tile_pool(name="ps", bufs=3, space="PSUM"))
        psum1_pool = ctx.enter_context(tc.tile_pool(name="ps1", bufs=1, space="PSUM"))
        dram_pool = ctx.enter_context(tc.tile_pool(name="dram", bufs=2, space="DRAM"))

        # constants (d first: first matmul needs it; edges last: needed latest)
        d_sb = const_pool.tile([128, 2, N], bf16)
        nc.sync.dma_start(d_sb[:], d_in[:])
        edges_sb = const_pool.tile([128, NPAIR, 2, 2, Q], i32)
        ones_sb = const_pool.tile([128, 1], f32)
        nc.vector.memset(ones_sb[:], 1.0)
        # per-batch partials: cols [0,BPC) = sum(w*g), cols [BPC,2*BPC) = sum(w)
        red_sb = const_pool.tile([128, 2 * BPC], f32)

        for t in range(NPAIR):
            # ---- load P^T for a batch pair (scalar queue: declustered from
            # the sync queue that issues spills)
            pt2 = pt_pool.tile([128, 2, 2, N], bf16)
            nc.scalar.dma_start(pt2[:], pt_in[t])
            if t == 0:
                nc.sync.dma_start(edges_sb[:], e_in[:])

            # ---- QT[n, (b2, i)] = sum_k D[k, n] * PT[k, (b2, i)]
            qt_sb = qt_pool.tile([128, 2, 2, N], bf16)  # (ncx, b2, i)
            for ncx in range(2):
                qt_ps = psum_pool.tile([128, 2, N], f32, tag="qtps")
                for kc in range(2):
                    nc.tensor.matmul(
                        qt_ps[:].rearrange("p a b -> p (a b)"),
                        lhsT=d_sb[:, kc, ncx * 128 : (ncx + 1) * 128],
                        rhs=pt2[:, kc, :, :].rearrange("p a b -> p (a b)"),
                        start=(kc == 0),
                        stop=(kc == 1),
                    )
                nc.scalar.copy(qt_sb[:, ncx], qt_ps[:])

            g2_sb = g_pool.tile([128, 2, 2, N], f32)  # (b2, ic, j)
            for b2 in range(2):
                # ---- G[(ic), j] = sum_n QT[n, i] * PT[n, j]
                g_ps = psum_pool.tile([128, 2, N], f32, tag="gps")  # (ic, j)
                for ic in range(2):
                    for ncx in range(2):
                        nc.tensor.matmul(
                            g_ps[:, ic, :],
                            lhsT=qt_sb[:, ncx, b2, ic * 128 : (ic + 1) * 128],
                            rhs=pt2[:, ncx, b2, :],
                            start=(ncx == 0),
                            stop=(ncx == 1),
                        )
                if b2 == 0:
                    nc.vector.tensor_copy(g2_sb[:, b2], g_ps[:])
                else:
                    nc.scalar.copy(g2_sb[:, b2], g_ps[:])

            # ---- spill both G tables; (b2, c, p, j) order == flat
            # f = b2*65536 + 256*i + j order
            g_d = dram_pool.tile([2, 2, 128, N], f32, tag="gd")
            nc.sync.dma_start(g_d.rearrange("b c p j -> p b c j"), g2_sb[:])

            # ---- gather per-edge values (one indirect DMA per batch; idx
            # stays 16-bit, the pair-table half is selected via element_offset)
            gsel = e_pool.tile([128, 2, Q], f32, tag="gsel")
            for b2 in range(2):
                nc.gpsimd.indirect_dma_start(
                    out=gsel[:, b2],
                    out_offset=None,
                    in_=g_d.rearrange("b c p (j u) -> (b c p j) u", u=1),
                    in_offset=bass.IndirectOffsetOnAxis(
                        ap=edges_sb[:, t, 0, b2], axis=0
                    ),
                    element_offset=b2 * (N * N),
                )

            # ---- per-batch partial sums
            ew_ap = edges_sb[:, t, 1].bitcast(f32)
            prod = e_pool.tile([128, 2, Q], f32, tag="prod")
            nc.vector.tensor_tensor(
                out=prod[:], in0=gsel[:], in1=ew_ap, op=mybir.AluOpType.mult
            )
            for b2 in range(2):
                b = 2 * t + b2
                nc.vector.tensor_reduce(
                    out=red_sb[:, b : b + 1],
                    in_=prod[:, b2, :],
                    axis=mybir.AxisListType.X,
                    op=mybir.AluOpType.add,
                )
                nc.vector.tensor_reduce(
                    out=red_sb[:, BPC + b : BPC + b + 1],
                    in_=ew_ap[:, b2, :],
                    axis=mybir.AxisListType.X,
                    op=mybir.AluOpType.add,
                )

        # ---- cross-partition reduce of all partials in one matmul
        red_ps = psum1_pool.tile([1, 2 * BPC], f32, tag="redps")
        nc.tensor.matmul(
            red_ps[:], lhsT=ones_sb[:], rhs=red_sb[:], start=True, stop=True
        )
        fin = const_pool.tile([1, 2 * BPC], f32)
        nc.vector.tensor_copy(fin[:], red_ps[:])
        nc.sync.dma_start(out[:], fin[:])

    if not nc.is_finalized():
        nc.finalize()
    return nc


_NC_CACHE = {}


def _get_nc():
    if "nc" not in _NC_CACHE:
        _NC_CACHE["nc"] = _build_bass()
    return _NC_CACHE["nc"]


def _prep_in_maps(P, d_error, edge_i, edge_j, edge_w):
    P = np.asarray(P, dtype=np.float32)
    d_error = np.asarray(d_error, dtype=np.float32)
    edge_i = np.asarray(edge_i, dtype=np.int32)
    edge_j = np.asarray(edge_j, dtype=np.int32)
    edge_w = np.asarray(edge_w, dtype=np.float32)

    # P^T pairs: pt[t, p, kc, b2, i] = P[2t+b2, i, kc*128+p]
    PT = np.ascontiguousarray(np.transpose(P, (0, 2, 1)))  # [B, N(k), N(i)]
    PT = PT.reshape(B // 2, 2, 2, 128, N).transpose(0, 3, 2, 1, 4)
    PT = np.ascontiguousarray(PT).astype(ml_dtypes.bfloat16)
    D = np.ascontiguousarray(d_error.reshape(2, 128, N).transpose(1, 0, 2))
    D = D.astype(ml_dtypes.bfloat16)

    # packed edges: [p, t, {fidx, w-bits}, b2, q]; edge e = q*128 + p at [p, q]
    fidx = (edge_i << 8) | edge_j  # [B, E] int32
    wbits = edge_w.view(np.int32)
    packed = np.stack([fidx, wbits], axis=1)  # [B, 2(fw), E]
    # -> [p, t, fw, b2, q]
    packed = packed.reshape(B // 2, 2, 2, Q, 128).transpose(4, 0, 2, 1, 3)
    packed = np.ascontiguousarray(packed)

    in_maps = []
    for c in range(NCORES):
        in_maps.append(
            {
                "pt": np.ascontiguousarray(PT[c * NPAIR : (c + 1) * NPAIR]),
                "derr": D,
                "edges": np.ascontiguousarray(
                    packed[:, c * NPAIR : (c + 1) * NPAIR]
                ),
            }
        )
    return in_maps


def run(P, d_error, edge_i, edge_j, edge_w, trace=False):
    """Run on 8 cores; returns (loss_scalar, BassKernelResults)."""
    nc = _get_nc()
    in_maps = _prep_in_maps(P, d_error, edge_i, edge_j, edge_w)
    res = run_bass_kernel_spmd(
        nc, in_maps, core_ids=list(range(NCORES)), trace=trace
    )
    # host-side all-reduce: loss = mean_b( sl_b / max(sw_b, 1e-8) )
    acc = 0.0
    for r in res.results:
        part = np.asarray(r["out"], dtype=np.float64).reshape(2 * BPC)
        sl, sw = part[:BPC], part[BPC:]
        acc += float(np.sum(sl / np.maximum(sw, 1e-8)))
    loss = np.float32(acc / B)
    return loss, res


def kernel(P, d_error, edge_i, edge_j, edge_w):
    loss, _ = run(P, d_error, edge_i, edge_j, edge_w, trace=False)
    return np.asarray(loss, dtype=np.float32)


# revision 13
# speedup vs baseline: 7.7313x; 1.0414x over previous
"""ErrorAwareEdgeLoss Trainium2 kernel.

Math: loss = mean_b [ (sum_e w_be * P[b,i_e,:] @ D @ P[b,j_e,:]) / max(sum_e w_be, 1e-8) ]

Reformulation:
    G_b = (P_b @ D) @ P_b^T            (two 256^3 matmuls on the PE, bf16)
    sum_e w_e * P[b,i_e,:] @ D @ P[b,j_e,:] = sum_e w_e * G_b[i_e, j_e]

Per-edge access path:
    G_b spills to DRAM as a flat [65536]-f32 table; one indirect DMA per
    batch (hardware dynamic DGE, per-element 16-bit offsets
    f = 256*i + j read from SBUF) fetches all 8192 per-edge values.

Schedule: QT = (P @ D)^T is computed for two batches per matmul (rhs 512
wide); all edge indices/weights load in one DMA up front; the per-batch
reduce is software-pipelined one batch behind its gather so no engine
queue ever blocks on an in-flight DMA.

Sharding: data-parallel over batch: 8 NeuronCores x 8 batches. Each core
emits per-sample partial sums (sum w*g and sum w per batch); the host
performs the final divide + mean (the all-reduce of the sharding hint).
"""

from contextlib import ExitStack

import ml_dtypes
import numpy as np

import concourse.bacc as bacc
import concourse.bass as bass
import concourse.mybir as mybir
import concourse.tile as tile
from concourse.bass_utils import run_bass_kernel_spmd

B, N, E = 64, 256, 8192
NCORES = 8
BPC = B // NCORES  # batches per core
NPAIR = BPC // 2
Q = E // 128  # edges per partition (64)

f32 = mybir.dt.float32
bf16 = mybir.dt.bfloat16
i32 = mybir.dt.int32


def _build_bass():
    nc = bacc.Bacc("TRN2", target_bir_lowering=False, debug=False)

    # pt[t, p, kc, b2, i] = P[2t+b2, i, kc*128+p]
    pt_in = nc.dram_tensor("pt", [NPAIR, 128, 2, 2, N], bf16, kind="ExternalInput")
    d_in = nc.dram_tensor("derr", [128, 2, N], bf16, kind="ExternalInput")
    # edges[p, t, 0, b2, q] = 256*i + j; edges[p, t, 1, b2, q] = bits(w)
    e_in = nc.dram_tensor("edges", [128, NPAIR, 2, 2, Q], i32, kind="ExternalInput")
    out = nc.dram_tensor("out", [1, 2 * BPC], f32, kind="ExternalOutput")

    with tile.TileContext(nc) as tc, ExitStack() as ctx:
        const_pool = ctx.enter_context(tc.tile_pool(name="const", bufs=1))
        pt_pool = ctx.enter_context(tc.tile_pool(name="pt", bufs=3))
        qt_pool = ctx.enter_context(tc.tile_pool(name="qt", bufs=2))
        g_pool = ctx.enter_context(tc.tile_pool(name="g", bufs=3))
        e_pool = ctx.enter_context(tc.tile_pool(name="edges", bufs=3))
        psum_pool = ctx.enter_context(tc.tile_pool(name="ps", bufs=3, space="PSUM"))
        psum1_pool = ctx.enter_context(tc.tile_pool(name="ps1", bufs=1, space="PSUM"))
        dram_pool = ctx.enter_context(tc.tile_pool(name="dram", bufs=3, space="DRAM"))

        # constants: edges first (largest slack), then d (first matmul input)
        edges_sb = const_pool.tile([128, NPAIR, 2, 2, Q], i32)
        nc.sync.dma_start(edges_sb[:], e_in[:])
        d_sb = const_pool.tile([128, 2, N], bf16)
        nc.sync.dma_start(d_sb[:], d_in[:])
        ones_sb = const_pool.tile([128, 1], f32)
        nc.vector.memset(ones_sb[:], 1.0)
        # per-batch partials: cols [0,BPC) = sum(w*g), cols [BPC,2*BPC) = sum(w)
        red_sb = const_pool.tile([128, 2 * BPC], f32)

        pending = None  # (gsel_ap, ew_ap, b) awaiting reduce — one batch behind

        def flush_pending():
            nonlocal pending
            if pending is None:
                return
            gsel_ap, ew_ap, b = pending
            prod = e_pool.tile([128, Q], f32, tag="prod")
            nc.vector.tensor_tensor(
                out=prod[:], in0=gsel_ap, in1=ew_ap, op=mybir.AluOpType.mult
            )
            nc.vector.tensor_reduce(
                out=red_sb[:, b : b + 1],
                in_=prod[:],
                axis=mybir.AxisListType.X,
                op=mybir.AluOpType.add,
            )
            pending = None

        for t in range(NPAIR):
            # ---- load P^T for a batch pair
            pt2 = pt_pool.tile([128, 2, 2, N], bf16)
            nc.sync.dma_start(pt2[:], pt_in[t])

            # ---- QT[n, (b2, i)] = sum_k D[k, n] * PT[k, (b2, i)]
            qt_sb = qt_pool.tile([128, 2, 2, N], bf16)  # (ncx, b2, i)
            for ncx in range(2):
                qt_ps = psum_pool.tile([128, 2, N], f32, tag="qtps")
                for kc in range(2):
                    nc.tensor.matmul(
                        qt_ps[:].rearrange("p a b -> p (a b)"),
                        lhsT=d_sb[:, kc, ncx * 128 : (ncx + 1) * 128],
                        rhs=pt2[:, kc, :, :].rearrange("p a b -> p (a b)"),
                        start=(kc == 0),
                        stop=(kc == 1),
                    )
                nc.scalar.copy(qt_sb[:, ncx], qt_ps[:])

            for b2 in range(2):
                b = 2 * t + b2
                # ---- G[(ic), j] = sum_n QT[n, i] * PT[n, j]
                g_ps = psum_pool.tile([128, 2, N], f32, tag="gps")  # (ic, j)
                for ic in range(2):
                    for ncx in range(2):
                        nc.tensor.matmul(
                            g_ps[:, ic, :],
                            lhsT=qt_sb[:, ncx, b2, ic * 128 : (ic + 1) * 128],
                            rhs=pt2[:, ncx, b2, :],
                            start=(ncx == 0),
                            stop=(ncx == 1),
                        )
                g_sb = g_pool.tile([128, 2, N], f32)
                nc.vector.tensor_copy(g_sb[:], g_ps[:])
                # sum(w) reduce for this batch: vector has slack here and the
                # input (edges_sb) loaded long ago
                nc.vector.tensor_reduce(
                    out=red_sb[:, BPC + b : BPC + b + 1],
                    in_=edges_sb[:, t, 1, b2].bitcast(f32),
                    axis=mybir.AxisListType.X,
                    op=mybir.AluOpType.add,
                )

                # ---- spill G; (c, p, j) order == flat f = 256*i + j order
                g_d = dram_pool.tile([2, 128, N], f32, tag="gd")
                nc.sync.dma_start(g_d.rearrange("c p j -> p c j"), g_sb[:])

                # ---- gather the 8192 per-edge values in one indirect DMA
                gsel = e_pool.tile([128, Q], f32, tag="gsel")
                nc.gpsimd.indirect_dma_start(
                    out=gsel[:],
                    out_offset=None,
                    in_=g_d.rearrange("c p (j u) -> (c p j) u", u=1),
                    in_offset=bass.IndirectOffsetOnAxis(
                        ap=edges_sb[:, t, 0, b2], axis=0
                    ),
                )

                # ---- reduce the PREVIOUS batch (its gather is long done)
                flush_pending()
                pending = (gsel[:], edges_sb[:, t, 1, b2].bitcast(f32), b)

        flush_pending()

        # ---- cross-partition reduce of all partials in one matmul
        red_ps = psum1_pool.tile([1, 2 * BPC], f32, tag="redps")
        nc.tensor.matmul(
            red_ps[:], lhsT=ones_sb[:], rhs=red_sb[:], start=True, stop=True
        )
        fin = const_pool.tile([1, 2 * BPC], f32)
        nc.vector.tensor_copy(fin[:], red_ps[:])
        nc.sync.dma_start(out[:], fin[:])

    if not nc.is_finalized():
        nc.finalize()
    return nc


_NC_CACHE = {}


def _get_nc():
    if "nc" not in _NC_CACHE:
        _NC_CACHE["nc"] = _build_bass()
    return _NC_CACHE["nc"]


def _prep_in_maps(P, d_error, edge_i, edge_j, edge_w):
    P = np.asarray(P, dtype=np.float32)
    d_error = np.asarray(d_error, dtype=np.float32)
    edge_i = np.asarray(edge_i, dtype=np.int32)
    edge_j = np.asarray(edge_j, dtype=np.int32)
    edge_w = np.asarray(edge_w, dtype=np.float32)

    # P^T pairs: pt[t, p, kc, b2, i] = P[2t+b2, i, kc*128+p]
    PT = np.ascontiguousarray(np.transpose(P, (0, 2, 1)))  # [B, N(k), N(i)]
    PT = PT.reshape(B // 2, 2, 2, 128, N).transpose(0, 3, 2, 1, 4)
    PT = np.ascontiguousarray(PT).astype(ml_dtypes.bfloat16)
    D = np.ascontiguousarray(d_error.reshape(2, 128, N).transpose(1, 0, 2))
    D = D.astype(ml_dtypes.bfloat16)

    # packed edges: [p, t, {fidx, w-bits}, b2, q]; edge e = q*128 + p at [p, q]
    fidx = (edge_i << 8) | edge_j  # [B, E] int32
    wbits = edge_w.view(np.int32)
    packed = np.stack([fidx, wbits], axis=1)  # [B, 2(fw), E]
    # -> [p, t, fw, b2, q]
    packed = packed.reshape(B // 2, 2, 2, Q, 128).transpose(4, 0, 2, 1, 3)
    packed = np.ascontiguousarray(packed)

    in_maps = []
    for c in range(NCORES):
        in_maps.append(
            {
                "pt": np.ascontiguousarray(PT[c * NPAIR : (c + 1) * NPAIR]),
                "derr": D,
                "edges": np.ascontiguousarray(
                    packed[:, c * NPAIR : (c + 1) * NPAIR]
                ),
            }
        )
    return in_maps


def run(P, d_error, edge_i, edge_j, edge_w, trace=False):
    """Run on 8 cores; returns (loss_scalar, BassKernelResults)."""
    nc = _get_nc()
    in_maps = _prep_in_maps(P, d_error, edge_i, edge_j, edge_w)
    res = run_bass_kernel_spmd(
        nc, in_maps, core_ids=list(range(NCORES)), trace=trace
    )
    # host-side all-reduce: loss = mean_b( sl_b / max(sw_b, 1e-8) )
    acc = 0.0
    for r in res.results:
        part = np.asarray(r["out"], dtype=np.float64).reshape(2 * BPC)
        sl, sw = part[:BPC], part[BPC:]
        acc += float(np.sum(sl / np.maximum(sw, 1e-8)))
    loss = np.float32(acc / B)
    return loss, res


def kernel(P, d_error, edge_i, edge_j, edge_w):
    loss, _ = run(P, d_error, edge_i, edge_j, edge_w, trace=False)
    return np.asarray(loss, dtype=np.float32)


# revision 16
# speedup vs baseline: 8.4359x; 1.0911x over previous
"""ErrorAwareEdgeLoss Trainium2 kernel.

Math: loss = mean_b [ (sum_e w_be * P[b,i_e,:] @ D @ P[b,j_e,:]) / max(sum_e w_be, 1e-8) ]

Reformulation:
    G_b = (P_b @ D) @ P_b^T            (two 256^3 matmuls on the PE, bf16)
    sum_e w_e * P[b,i_e,:] @ D @ P[b,j_e,:] = sum_e w_e * G_b[i_e, j_e]

Per-edge access path:
    Both G tables of a batch pair spill to one DRAM table viewed as
    [65536, 2] f32; one indirect DMA per pair (hardware dynamic DGE)
    fetches 2 consecutive f32 per edge at 16-bit offset
    idx = b2*32768 + (f>>1), f = 256*i + j. The odd/even lane select is
    folded into host-interleaved weights (w at lane f&1, 0 at the other),
    so prod-and-reduce needs one multiply + one reduce per batch.

Sharding: data-parallel over batch: 8 NeuronCores x 8 batches. Each core
emits per-sample partial sums (sum w*g and sum w per batch); the host
performs the final divide + mean (the all-reduce of the sharding hint).
"""

from contextlib import ExitStack

import ml_dtypes
import numpy as np

import concourse.bacc as bacc
import concourse.bass as bass
import concourse.mybir as mybir
import concourse.tile as tile
from concourse.bass_utils import run_bass_kernel_spmd

B, N, E = 64, 256, 8192
NCORES = 8
BPC = B // NCORES  # batches per core
NPAIR = BPC // 2
Q = E // 128  # edges per partition (64)

f32 = mybir.dt.float32
bf16 = mybir.dt.bfloat16
i32 = mybir.dt.int32


def _build_bass():
    nc = bacc.Bacc("TRN2", target_bir_lowering=False, debug=False)

    # pt[t, p, kc, b2, i] = P[2t+b2, i, kc*128+p]
    pt_in = nc.dram_tensor("pt", [NPAIR, 128, 2, 2, N], bf16, kind="ExternalInput")
    d_in = nc.dram_tensor("derr", [128, 2, N], bf16, kind="ExternalInput")
    # eidx[p, t, b2, q] = b2*32768 + (256*i + j)>>1
    ei_in = nc.dram_tensor("eidx", [128, NPAIR, 2, Q], i32, kind="ExternalInput")
    # ew2[p, t, b2, q, l] = w if l == (256*i + j)&1 else 0
    ew_in = nc.dram_tensor("ew2", [128, NPAIR, 2, Q, 2], f32, kind="ExternalInput")
    out = nc.dram_tensor("out", [1, 2 * BPC], f32, kind="ExternalOutput")

    with tile.TileContext(nc) as tc, ExitStack() as ctx:
        const_pool = ctx.enter_context(tc.tile_pool(name="const", bufs=1))
        pt_pool = ctx.enter_context(tc.tile_pool(name="pt", bufs=3))
        qt_pool = ctx.enter_context(tc.tile_pool(name="qt", bufs=2))
        g_pool = ctx.enter_context(tc.tile_pool(name="g", bufs=3))
        e_pool = ctx.enter_context(tc.tile_pool(name="edges", bufs=3))
        psum_pool = ctx.enter_context(tc.tile_pool(name="ps", bufs=3, space="PSUM"))
        psum1_pool = ctx.enter_context(tc.tile_pool(name="ps1", bufs=1, space="PSUM"))
        dram_pool = ctx.enter_context(tc.tile_pool(name="dram", bufs=2, space="DRAM"))

        # inputs on the scalar queue: the sync queue carries only spills, so
        # a spill's data never waits behind bulk input transfers
        d_sb = const_pool.tile([128, 2, N], bf16)
        nc.scalar.dma_start(d_sb[:], d_in[:])
        eidx_sb = const_pool.tile([128, NPAIR, 2, Q], i32)
        ew2_sb = const_pool.tile([128, NPAIR, 2, Q, 2], f32)
        ones_sb = const_pool.tile([128, 1], f32)
        nc.vector.memset(ones_sb[:], 1.0)
        # per-batch partials: cols [0,BPC) = sum(w*g), cols [BPC,2*BPC) = sum(w)
        red_sb = const_pool.tile([128, 2 * BPC], f32)

        pending = None  # (gsel tile, t) awaiting reduce — one pair behind

        def flush_pending():
            nonlocal pending
            if pending is None:
                return
            gsel, t = pending
            prod = e_pool.tile([128, 2, Q, 2], f32, tag="prod")
            nc.vector.tensor_tensor(
                out=prod[:], in0=gsel[:], in1=ew2_sb[:, t], op=mybir.AluOpType.mult
            )
            for b2 in range(2):
                b = 2 * t + b2
                nc.vector.tensor_reduce(
                    out=red_sb[:, b : b + 1],
                    in_=prod[:, b2].rearrange("p a b -> p (a b)"),
                    axis=mybir.AxisListType.X,
                    op=mybir.AluOpType.add,
                )
            pending = None

        for t in range(NPAIR):
            # ---- load P^T for a batch pair
            pt2 = pt_pool.tile([128, 2, 2, N], bf16)
            nc.scalar.dma_start(pt2[:], pt_in[t])
            if t == 0:
                # after d/pt0 in the scalar queue; before any edge reads
                nc.scalar.dma_start(eidx_sb[:], ei_in[:])
                nc.scalar.dma_start(ew2_sb[:], ew_in[:])

            # ---- QT[n, (b2, i)] = sum_k D[k, n] * PT[k, (b2, i)]
            qt_sb = qt_pool.tile([128, 2, 2, N], bf16)  # (ncx, b2, i)
            for ncx in range(2):
                qt_ps = psum_pool.tile([128, 2, N], f32, tag="qtps")
                for kc in range(2):
                    nc.tensor.matmul(
                        qt_ps[:].rearrange("p a b -> p (a b)"),
                        lhsT=d_sb[:, kc, ncx * 128 : (ncx + 1) * 128],
                        rhs=pt2[:, kc, :, :].rearrange("p a b -> p (a b)"),
                        start=(kc == 0),
                        stop=(kc == 1),
                    )
                nc.scalar.copy(qt_sb[:, ncx], qt_ps[:])

            g_d = dram_pool.tile([2, 2, 128, N], f32, tag="gd")  # (b2, c, p, j)
            for b2 in range(2):
                # ---- G[(ic), j] = sum_n QT[n, i] * PT[n, j]
                g_ps = psum_pool.tile([128, 2, N], f32, tag="gps")  # (ic, j)
                for ic in range(2):
                    for ncx in range(2):
                        nc.tensor.matmul(
                            g_ps[:, ic, :],
                            lhsT=qt_sb[:, ncx, b2, ic * 128 : (ic + 1) * 128],
                            rhs=pt2[:, ncx, b2, :],
                            start=(ncx == 0),
                            stop=(ncx == 1),
                        )
                g_sb = g_pool.tile([128, 2, N], f32)
                if b2 == 0:
                    nc.vector.tensor_copy(g_sb[:], g_ps[:])
                else:
                    nc.scalar.copy(g_sb[:], g_ps[:])
                # sum(w) reduce for this batch: vector has slack here
                b = 2 * t + b2
                nc.vector.tensor_reduce(
                    out=red_sb[:, BPC + b : BPC + b + 1],
                    in_=ew2_sb[:, t, b2].rearrange("p a b -> p (a b)"),
                    axis=mybir.AxisListType.X,
                    op=mybir.AluOpType.add,
                )
                # ---- spill this batch's G into its half of the pair table
                nc.sync.dma_start(
                    g_d[b2].rearrange("c p j -> p c j"), g_sb[:]
                )

            # ---- gather 2 consecutive f32 per edge, both batches in one
            # indirect DMA (16-bit offsets: b2*32768 + f>>1)
            gsel = e_pool.tile([128, 2, Q, 2], f32, tag="gsel")
            nc.gpsimd.indirect_dma_start(
                out=gsel[:].rearrange("p a b c -> p (a b c)"),
                out_offset=None,
                in_=g_d.rearrange("b c p (j2 u) -> (b c p j2) u", u=2),
                in_offset=bass.IndirectOffsetOnAxis(ap=eidx_sb[:, t], axis=0),
            )

            # ---- reduce the PREVIOUS pair (its gather is long done)
            flush_pending()
            pending = (gsel, t)

        flush_pending()

        # ---- cross-partition reduce of all partials in one matmul
        red_ps = psum1_pool.tile([1, 2 * BPC], f32, tag="redps")
        nc.tensor.matmul(
            red_ps[:], lhsT=ones_sb[:], rhs=red_sb[:], start=True, stop=True
        )
        fin = const_pool.tile([1, 2 * BPC], f32)
        nc.vector.tensor_copy(fin[:], red_ps[:])
        nc.sync.dma_start(out[:], fin[:])

    if not nc.is_finalized():
        nc.finalize()
    return nc


_NC_CACHE = {}


def _get_nc():
    if "nc" not in _NC_CACHE:
        _NC_CACHE["nc"] = _build_bass()
    return _NC_CACHE["nc"]


def _prep_in_maps(P, d_error, edge_i, edge_j, edge_w):
    P = np.asarray(P, dtype=np.float32)
    d_error = np.asarray(d_error, dtype=np.float32)
    edge_i = np.asarray(edge_i, dtype=np.int32)
    edge_j = np.asarray(edge_j, dtype=np.int32)
    edge_w = np.asarray(edge_w, dtype=np.float32)

    # P^T pairs: pt[t, p, kc, b2, i] = P[2t+b2, i, kc*128+p]
    PT = np.ascontiguousarray(np.transpose(P, (0, 2, 1)))  # [B, N(k), N(i)]
    PT = PT.reshape(B // 2, 2, 2, 128, N).transpose(0, 3, 2, 1, 4)
    PT = np.ascontiguousarray(PT).astype(ml_dtypes.bfloat16)
    D = np.ascontiguousarray(d_error.reshape(2, 128, N).transpose(1, 0, 2))
    D = D.astype(ml_dtypes.bfloat16)

    # gather index: b2*32768 + f>>1 (pair-table element-pair offset)
    f = (edge_i << 8) | edge_j  # [B, E] int32
    b_off = (np.arange(B, dtype=np.int32) & 1)[:, None] << 15
    idx = b_off | (f >> 1)
    # lane-selected weights: w at lane f&1, 0 at the other
    lane = (f & 1)[..., None]  # [B, E, 1]
    w2 = edge_w[..., None] * (lane == np.arange(2)).astype(np.float32)  # [B, E, 2]

    # edge e = q*128 + p at [p, q]
    # idx -> [p, t, b2, q]
    idx_l = idx.reshape(B // 2, 2, Q, 128).transpose(3, 0, 1, 2)
    idx_l = np.ascontiguousarray(idx_l)
    # w2 -> [p, t, b2, q, l]
    w2_l = w2.reshape(B // 2, 2, Q, 128, 2).transpose(3, 0, 1, 2, 4)
    w2_l = np.ascontiguousarray(w2_l)

    in_maps = []
    for c in range(NCORES):
        sl = slice(c * NPAIR, (c + 1) * NPAIR)
        in_maps.append(
            {
                "pt": np.ascontiguousarray(PT[sl]),
                "derr": D,
                "eidx": np.ascontiguousarray(idx_l[:, sl]),
                "ew2": np.ascontiguousarray(w2_l[:, sl]),
            }
        )
    return in_maps


def run(P, d_error, edge_i, edge_j, edge_w, trace=False):
    """Run on 8 cores; returns (loss_scalar, BassKernelResults)."""
    nc = _get_nc()
    in_maps = _prep_in_maps(P, d_error, edge_i, edge_j, edge_w)
    res = run_bass_kernel_spmd(
        nc, in_maps, core_ids=list(range(NCORES)), trace=trace
    )
    # host-side all-reduce: loss = mean_b( sl_b / max(sw_b, 1e-8) )
    acc = 0.0
    for r in res.results:
        part = np.asarray(r["out"], dtype=np.float64).reshape(2 * BPC)
        sl, sw = part[:BPC], part[BPC:]
        acc += float(np.sum(sl / np.maximum(sw, 1e-8)))
    loss = np.float32(acc / B)
    return loss, res


def kernel(P, d_error, edge_i, edge_j, edge_w):
    loss, _ = run(P, d_error, edge_i, edge_j, edge_w, trace=False)
    return np.asarray(loss, dtype=np.float32)
